# revision 8
# baseline (speedup 1.0000x reference)
"""CapsuleConv2d (3-iteration dynamic routing) Bass kernel for 8 TRN2 cores.

Strategy (data-parallel over batch, 2 images per core):
  - priors computed by PE in fp16 (stationary = padded-x window, moving =
    structured weight constants); PSUM fp32.
  - P staged to SBUF twice by ACT (fp16): Pu in (ij,o,f,u) order (u
    innermost) for the H = P*v multiplies, Pf in (o,u,ij,f) order
    ((ij,f) innermost) for the G = E'*P multiplies.  All big DVE
    multiplies run in 2x_1p mode (2-byte dtypes, innermost stride 1).
  - reductions: DVE TensorReduce has no fast mode (1 elem/cycle any
    dtype), so the big contractions are pairwise ADD TREES of packed
    fp16 tensor_tensor ops, which do hit 2x mode: a 9216-col direct
    reduce becomes ~2300 equivalent cycles.  Final tree level outputs
    fp32 (b, s, Z accumulate in fp32 for precision).
  - E = exp(b) is bf16 (fp16 would overflow: b can reach ~30); the
    normalized E' = E/Z is fp16 (values <= 1), G/H/v fp16.
"""
import numpy as np

import concourse.bass as bass
import concourse.bacc as bacc
import concourse.tile as tile
import concourse.mybir as mybir
import concourse.bass_utils as bass_utils

# All ACT functions we use (Exp, Ln, Square, Copy, ...) live together in the
# "natural_log_exp_and_others" table set, but bacc's table-load pass picks a
# per-function set greedily (Ln -> natural_log, Exp -> exp_and_others),
# thrashing ~2.7us table loads between them.  Restrict Exp/Ln to the combined
# set so a single load covers the whole kernel.
_orig_get_tables = bacc.get_activation_tables
_AFT = mybir.ActivationFunctionType


def _patched_get_tables(arch):
    tables = dict(_orig_get_tables(arch))
    for name, funcs in tables.items():
        if name != "natural_log_exp_and_others":
            tables[name] = funcs - {_AFT.Exp, _AFT.Ln}
    return tables


bacc.get_activation_tables = _patched_get_tables

# ---- problem constants (hardcoded; must match setup_inputs) ----
O, F, U, D = 4, 4, 8, 8
KH = KW = 3
NIJ = KH * KW
H = W = 64
C = 32
N_FULL = 16
N_CORES = 8
IMG_PER_CORE = N_FULL // N_CORES
HP, WP = H + 2, W + 2              # padded input
LT_ROWS = 2                        # output rows per 128-loc tile
NLT = H // LT_ROWS                 # 32 loc-tiles per image
ST_LT = 4                          # loc-tiles per super-tile (512 locs)
NST = NLT // ST_LT                 # 8 super-tiles per image
PB = 2                             # super-tiles batched per routing pass
PLT = PB * ST_LT                   # loc-tiles per routing pass (8)
EPS = 1e-12
KK = ST_LT * NIJ                   # collapsed (lt, ij) per half
MH = ST_LT * 32                    # (lt, o, u) per half = 128
BH = ST_LT * 144                   # (lt, ij, o, f) per half = 576

f32 = mybir.dt.float32
f16 = mybir.dt.float16
bf16 = mybir.dt.bfloat16
AL = mybir.AluOpType
AF = mybir.ActivationFunctionType
AX = mybir.AxisListType

_COMPILED = None


def _build(dump=False, repeat=1):
    nc = bacc.Bacc("TRN2", target_bir_lowering=False, debug=False)

    xin_d = nc.dram_tensor("xin", [IMG_PER_CORE, C, H * W], f16,
                           kind="ExternalInput").ap()
    wmov_d = nc.dram_tensor("wmov", [C, NIJ * 128], f16,
                            kind="ExternalInput").ap()
    wsum_d = nc.dram_tensor("wsum", [C, NIJ * 32], f16,
                            kind="ExternalInput").ap()
    ident_d = nc.dram_tensor("ident", [128, 128], f16,
                             kind="ExternalInput").ap()
    out_d = nc.dram_tensor("out", [IMG_PER_CORE, C, H * W], f32,
                           kind="ExternalOutput").ap()

    with tile.TileContext(nc) as tc:
        with tc.tile_pool(name="const", bufs=1) as cpool, \
             tc.tile_pool(name="xpad", bufs=1) as xpool, \
             tc.tile_pool(name="stage", bufs=1) as spool, \
             tc.tile_pool(name="pu", bufs=4) as pupool, \
             tc.tile_pool(name="pf", bufs=4) as pfpool, \
             tc.tile_pool(name="gh", bufs=2) as ghpool, \
             tc.tile_pool(name="small", bufs=2) as smpool, \
             tc.tile_pool(name="tree", bufs=1) as trpool, \
             tc.tile_pool(name="ppri", bufs=2, space="PSUM") as ppri, \
             tc.tile_pool(name="ps0", bufs=1, space="PSUM") as ps0, \
             tc.tile_pool(name="ptp", bufs=1, space="PSUM") as ptp:

            wmov_s = cpool.tile([C, NIJ * 128], f16, tag="wmov")
            wsum_s = cpool.tile([C, NIJ * 32], f16, tag="wsum")
            ident_s = cpool.tile([128, 128], f16, tag="ident")
            eps_s = cpool.tile([128, 1], f32, tag="eps")
            one_s = cpool.tile([128, 1], f32, tag="one")
            nc.sync.dma_start(wmov_s[:], wmov_d[:])
            nc.sync.dma_start(wsum_s[:], wsum_d[:])
            nc.sync.dma_start(ident_s[:], ident_d[:])
            nc.gpsimd.memset(eps_s[:], EPS)
            nc.gpsimd.memset(one_s[:], 1.0)

            for img in range(IMG_PER_CORE):
                xp = xpool.tile([C, HP * WP], f16, tag="xpad")
                nc.gpsimd.memset(xp[:], 0.0)
                xv = xp[:].rearrange("p (h w) -> p h w", h=HP, w=WP)
                nc.sync.dma_start(
                    xv[:, 1:1 + H, 1:1 + W],
                    xin_d[img].rearrange("p (h w) -> p h w", h=H, w=W))
                stage = spool.tile([C, H * W], f32, tag="stage")
                for pr_rep in range((NST // PB) * repeat):
                    pr = pr_rep % (NST // PB)
                    # ---- priors for PB super-tiles (PE, fp16) ----
                    Pu_sts, Pf_sts = [], []
                    s0_st = smpool.tile([128, PLT * 32], f32, tag="s0")
                    for half in range(PB):
                        st = pr * PB + half
                        Pu_st = pupool.tile([128, ST_LT * 1152], f16,
                                            tag="Pu")
                        Pf_st = pfpool.tile([128, ST_LT * 1152], f16,
                                            tag="Pf")
                        Pu_sts.append(Pu_st)
                        Pf_sts.append(Pf_st)
                        for lt in range(ST_LT):
                            r0 = (st * ST_LT + lt) * LT_ROWS
                            glt = half * ST_LT + lt
                            pp = ppri.tile([128, 1152], f32, tag="ppri")
                            s0p = ps0.tile([128, 32], f32, tag="s0p")
                            for ij in range(NIJ):
                                i, j = ij // KW, ij % KW
                                for r in range(LT_ROWS):
                                    xw = xv[:, r0 + i + r, j:j + W]
                                    prow = slice(r * W, (r + 1) * W)
                                    nc.tensor.matmul(
                                        pp[prow, ij * 128:(ij + 1) * 128],
                                        xw,
                                        wmov_s[:, ij * 128:(ij + 1) * 128],
                                        start=True, stop=True)
                                    nc.tensor.matmul(
                                        s0p[prow], xw,
                                        wsum_s[:, ij * 32:(ij + 1) * 32],
                                        start=(ij == 0),
                                        stop=(ij == NIJ - 1))
                            # PSUM pp is (f,o,u) per tap (wmov col
                            # order), so Pu (ij,f,o,u) is a straight copy.
                            nc.scalar.copy(
                                Pu_st[:, lt * 1152:(lt + 1) * 1152], pp[:])
                            # Pf: (o,u,ij,f); one ACT copy per o keeps the
                            # transposed access pattern at 3 free dims.
                            pfs = Pf_st[:, lt * 1152:(lt + 1) * 1152]
                            ppv = pp[:].rearrange(
                                "p (ij f o u) -> p ij f o u", ij=NIJ, f=F,
                                o=O, u=U)
                            pfv = pfs.rearrange(
                                "p (o u ij f) -> p o u ij f", o=O, u=U,
                                ij=NIJ, f=F)
                            for o_ in range(O):
                                nc.scalar.copy(
                                    pfv[:, o_].transpose([0, 1, 2, 3]),
                                    ppv[:, :, :, o_].transpose([0, 3, 1, 2]))
                            nc.scalar.copy(
                                s0_st[:, glt * 32:(glt + 1) * 32], s0p[:])

                    # ------- routing on this super-tile pair -------
                    def squash(s_st, tagp):
                        # s_st: [128, (lt, o, u)] fp32; returns v fp16.
                        # fi = n2/((1+n2)*sqrt(n2+eps)) computed as
                        # exp(0.5*ln(n2+eps) - ln(n2+1)) ... wait:
                        # fi = exp(ln n2 - ln(1+n2) - 0.5 ln(n2+eps))
                        #    = exp(0.5*ln(n2+eps) - ln(1+n2))  [n2 >> eps]
                        sq = smpool.tile([128, PLT * 32], f32,
                                         tag=f"sq{tagp}")
                        nc.scalar.activation(sq[:], s_st[:], AF.Square)
                        n2 = smpool.tile([128, PLT * O], f32,
                                         tag=f"n2{tagp}")
                        nc.vector.tensor_reduce(
                            n2[:],
                            sq[:].rearrange("p (lt o u) -> p lt o u",
                                            lt=PLT, o=O, u=U),
                            AX.X, AL.add)
                        Ltile = smpool.tile([128, PLT * O], f32,
                                            tag=f"L{tagp}")
                        nc.scalar.activation(Ltile[:], n2[:], AF.Ln,
                                             bias=eps_s[:])
                        Lp = smpool.tile([128, PLT * O], f32,
                                         tag=f"Lp{tagp}")
                        nc.scalar.activation(Lp[:], n2[:], AF.Ln,
                                             bias=one_s[:])
                        d_ = smpool.tile([128, PLT * O], f32,
                                         tag=f"d{tagp}")
                        nc.vector.scalar_tensor_tensor(
                            d_[:], Ltile[:], 0.5, Lp[:], AL.mult,
                            AL.subtract)
                        fi = smpool.tile([128, PLT * O], f32,
                                         tag=f"fi{tagp}")
                        nc.scalar.activation(fi[:], d_[:], AF.Exp)
                        v = smpool.tile([128, PLT * 32], f16,
                                        tag=f"v{tagp}")
                        fib = fi[:].rearrange("p (lt o) -> p lt o",
                                              lt=PLT).unsqueeze(3)
                        nc.vector.tensor_tensor(
                            v[:].rearrange("p (lt o u) -> p lt o u",
                                           lt=PLT, o=O, u=U),
                            s_st[:].rearrange("p (lt o u) -> p lt o u",
                                              lt=PLT, o=O, u=U),
                            fib.broadcast_to((128, PLT, O, U)), AL.mult)
                        return v

                    v = squash(s0_st, "0")

                    b_st = smpool.tile([128, PLT * 144], f32, tag="b")
                    hred = smpool.tile([128, PLT * 144], f32, tag="hred")
                    for it in range(3):
                        if it > 0:
                            # E = exp(b) bf16; Z = sum_o E (add tree);
                            # E' = E * (1/Z) -> fp16
                            # E = exp(b): b is (lt,ij,f,o); write E
                            # as (lt,o,ij,f) per loc-tile (3-dim APs)
                            E = smpool.tile([128, PLT * 144], f32, tag="E")
                            for glt in range(PLT):
                                bl = b_st[:, glt * 144:(glt + 1) *
                                          144].rearrange(
                                    "p (ij f o) -> p ij f o", ij=NIJ, f=F,
                                    o=O)
                                el = E[:, glt * 144:(glt + 1) *
                                       144].rearrange(
                                    "p (o ij f) -> p o ij f", o=O, ij=NIJ,
                                    f=F)
                                nc.scalar.activation(
                                    el, bl.transpose([0, 3, 1, 2]), AF.Exp)
                            # Z tree: sum over o (stride 36 per lt)
                            E2 = E[:].rearrange("p (lt t) -> p lt t",
                                                lt=PLT, t=144)
                            zt1 = trpool.tile([128, PLT * 72], f32,
                                              tag="zt1")
                            zt1v = zt1[:].rearrange("p (lt t) -> p lt t",
                                                    lt=PLT, t=72)
                            nc.vector.tensor_tensor(
                                zt1v, E2[:, :, 0:72], E2[:, :, 72:144],
                                AL.add)
                            Z = smpool.tile([128, PLT * 36], f32, tag="Z")
                            nc.vector.tensor_tensor(
                                Z[:].rearrange("p (lt t) -> p lt t",
                                               lt=PLT, t=36),
                                zt1v[:, :, 0:36], zt1v[:, :, 36:72],
                                AL.add)
                            Zi = smpool.tile([128, PLT * 36], f32,
                                             tag="Zi")
                            nc.vector.reciprocal(Zi[:], Z[:])
                            # E' = E * (1/Z) -> fp16 (fp32-rate op)
                            Ep = smpool.tile([128, PLT * 144], f16,
                                             tag="Ep")
                            Zib = Zi[:].rearrange(
                                "p (lt t) -> p lt t",
                                lt=PLT).unsqueeze(2).broadcast_to(
                                    (128, PLT, O, 36))
                            nc.vector.tensor_tensor(
                                Ep[:].rearrange("p (lt o t) -> p lt o t",
                                                lt=PLT, o=O, t=36),
                                E[:].rearrange("p (lt o t) -> p lt o t",
                                               lt=PLT, o=O, t=36),
                                Zib, AL.mult)
                            s_st = smpool.tile([128, PLT * 32], f32,
                                               tag="s")
                            for half in range(PB):
                                # G = E' * Pf (fp16, 2x), per loc-tile
                                G = ghpool.tile([128, ST_LT * 1152], f16,
                                                tag="gg")
                                for lt in range(ST_LT):
                                    glt = half * ST_LT + lt
                                    Gv = G[:, lt * 1152:(lt + 1) *
                                           1152].rearrange(
                                        "p (o u t) -> p o u t", o=O, u=U,
                                        t=36)
                                    Pfv = Pf_sts[half][:, lt * 1152:
                                                       (lt + 1) *
                                                       1152].rearrange(
                                        "p (o u t) -> p o u t", o=O, u=U,
                                        t=36)
                                    Eb = Ep[:, glt * 144:(glt + 1) *
                                            144].rearrange(
                                        "p (o t) -> p o t",
                                        o=O).unsqueeze(2).broadcast_to(
                                            (128, O, U, 36))
                                    nc.vector.tensor_tensor(Gv, Pfv, Eb,
                                                            AL.mult)
                                # s tree over (ij,f)=36 per (lt,o,u)
                                Gt = G[:].rearrange("p (M t) -> p M t",
                                                    M=MH, t=36)
                                st1 = trpool.tile([128, MH * 18], f16,
                                                  tag="st1")
                                s1v = st1[:].rearrange(
                                    "p (M t) -> p M t", M=MH, t=18)
                                nc.vector.tensor_tensor(
                                    s1v, Gt[:, :, 0:18], Gt[:, :, 18:36],
                                    AL.add)
                                st2 = trpool.tile([128, MH * 9], f16,
                                                  tag="st2")
                                s2v = st2[:].rearrange(
                                    "p (M t) -> p M t", M=MH, t=9)
                                nc.vector.tensor_tensor(
                                    s2v, s1v[:, :, 0:9], s1v[:, :, 9:18],
                                    AL.add)
                                st3 = trpool.tile([128, MH * 4], f16,
                                                  tag="st3")
                                s3v = st3[:].rearrange(
                                    "p (M t) -> p M t", M=MH, t=4)
                                nc.vector.tensor_tensor(
                                    s3v, s2v[:, :, 0:4], s2v[:, :, 4:8],
                                    AL.add)
                                st4 = trpool.tile([128, MH * 2], f16,
                                                  tag="st4")
                                s4v = st4[:].rearrange(
                                    "p (M t) -> p M t", M=MH, t=2)
                                nc.vector.tensor_tensor(
                                    s4v, s3v[:, :, 0:2], s3v[:, :, 2:4],
                                    AL.add)
                                st5 = trpool.tile([128, MH], f16,
                                                  tag="st5")
                                nc.vector.tensor_tensor(
                                    st5[:], s4v[:, :, 0], s4v[:, :, 1],
                                    AL.add)
                                # + carry (ij,f idx 8 of 0..8 nines)
                                nc.vector.tensor_tensor(
                                    s_st[:, half * MH:(half + 1) * MH],
                                    st5[:], s2v[:, :, 8], AL.add)
                            v = squash(s_st, "12")
                        if it < 2:
                            # b += sum_u Pu * v   (H fp16 2x; add tree)
                            dst = b_st if it == 0 else hred
                            for half in range(PB):
                                Hst = ghpool.tile([128, ST_LT * 1152], f16,
                                                  tag="hh")
                                H5 = Hst[:].rearrange(
                                    "p (lt k t) -> p lt k t", lt=ST_LT,
                                    k=NIJ * F, t=32)
                                Pu5 = Pu_sts[half][:].rearrange(
                                    "p (lt k t) -> p lt k t", lt=ST_LT,
                                    k=NIJ * F, t=32)
                                vb = v[:, half * 128:(half + 1) *
                                       128].rearrange(
                                    "p (lt t) -> p lt t",
                                    lt=ST_LT).unsqueeze(2).broadcast_to(
                                        (128, ST_LT, NIJ * F, 32))
                                nc.vector.tensor_tensor(H5, Pu5, vb,
                                                        AL.mult)
                                Hk = Hst[:].rearrange(
                                    "p (m u) -> p m u", m=BH, u=U)
                                bt1 = trpool.tile([128, BH * 4], f16,
                                                  tag="bt1")
                                b1v = bt1[:].rearrange(
                                    "p (m t) -> p m t", m=BH, t=4)
                                nc.vector.tensor_tensor(
                                    b1v, Hk[:, :, 0:4], Hk[:, :, 4:8],
                                    AL.add)
                                bt2 = trpool.tile([128, BH * 2], f32,
                                                  tag="bt2")
                                b2v = bt2[:].rearrange(
                                    "p (m t) -> p m t", m=BH, t=2)
                                nc.vector.tensor_tensor(
                                    b2v, b1v[:, :, 0:2], b1v[:, :, 2:4],
                                    AL.add)
                                nc.vector.tensor_tensor(
                                    dst[:, half * BH:(half + 1) * BH],
                                    b2v[:, :, 0], b2v[:, :, 1], AL.add)
                            if it == 1:
                                nc.vector.tensor_tensor(b_st[:], b_st[:],
                                                        hred[:], AL.add)

                    # v (fp16) -> transpose to [32, locs] & stage fp32
                    for glt in range(PLT):
                        r0 = (pr * PLT + glt) * LT_ROWS
                        tp = ptp.tile([32, 128], f16, tag="tp")
                        nc.tensor.transpose(tp[:],
                                            v[:, glt * 32:(glt + 1) * 32],
                                            ident_s[:])
                        nc.scalar.copy(
                            stage[:, r0 * W:r0 * W + LT_ROWS * W], tp[:])

                nc.sync.dma_start(out_d[img], stage[:])

    nc.compile()
    return nc


def _get_compiled():
    global _COMPILED
    if _COMPILED is None:
        _COMPILED = _build()
    return _COMPILED


def _make_consts(weight):
    w = np.asarray(weight, dtype=np.float32)  # [o, f, i, j, u, d]
    wmov = np.zeros((C, NIJ * 128), dtype=np.float16)
    wsum = np.zeros((C, NIJ * 32), dtype=np.float16)
    for o in range(O):
        for f in range(F):
            for ij in range(NIJ):
                i, j = ij // KW, ij % KW
                for u in range(U):
                    for d in range(D):
                        wmov[f * D + d,
                             ij * 128 + f * 32 + o * 8 + u] = w[o, f, i, j,
                                                                u, d]
                        wsum[f * D + d,
                             ij * 32 + o * 8 + u] = 0.25 * w[o, f, i, j, u,
                                                             d]
    return wmov, wsum


def make_in_maps(x, weight):
    x16 = np.asarray(x).astype(np.float16)
    wmov, wsum = _make_consts(weight)
    ident = np.eye(128, dtype=np.float16)
    in_maps = []
    for c in range(N_CORES):
        xin = x16[c * IMG_PER_CORE:(c + 1) * IMG_PER_CORE].reshape(
            IMG_PER_CORE, C, H * W)
        in_maps.append({
            "xin": np.ascontiguousarray(xin),
            "wmov": wmov,
            "wsum": wsum,
            "ident": ident,
        })
    return in_maps


def kernel(x, weight):
    nc = _get_compiled()
    in_maps = make_in_maps(x, weight)
    res = bass_utils.run_bass_kernel_spmd(nc, in_maps,
                                          core_ids=list(range(N_CORES)))
    out = np.empty((N_FULL, C, H, W), dtype=np.float32)
    for c in range(N_CORES):
        out[c * IMG_PER_CORE:(c + 1) * IMG_PER_CORE] = res.results[c][
            "out"].reshape(IMG_PER_CORE, C, H, W)
    return out
